# revision 14
# baseline (speedup 1.0000x reference)
"""Batch-hard triplet loss on 8 Trainium2 NeuronCores.

Data-parallel over rows (512 rows/core, 4 chunks of 128). The batch is
label-sorted on the host and each (core, chunk) gets a rotated view of
the embedding table (local col j <-> global (j + c*512 - 32 + m*128)):
all same-label columns of chunk row p land in [p+32-B, p+32+B], B<=32
(host-asserted; actual ~15), so the mask matmul covers local cols
[0, 512) and the hardest-positive scan only cols [0, 256).

Per 128-row chunk the PE accumulates, in two 4-bank fp32 PSUM groups,
    v(p, j) = x_p . x_j - ||x_j||^2/2 - (BIG/2) * [lab_p == lab_j]
(so d2 = -2v + ||x_p||^2; the -2 and + ||x_p||^2 run on the host).
Matmuls are grouped by stationary operand - per chunk: 1 mask MM (LH4,
one-hot * -BIG/2 + norm rows), 7 K=2 norm MMs (ones @ -sq/2 hi/lo
rows), then 8 main MMs (stationary = the chunk's own 128 columns of
XT2) - only 2 weight switches, which keeps the PE dense so the HAM
clock-gate upshifts 1.2 -> 2.4 GHz (junk matmuls during the initial
DMA wait pre-warm it).

ScalarE evacuates each PSUM group to fp16 SBUF ([128,2048] ACTIVATE
copies, double-buffered); VectorE reduces the fp16 block with a MAX
tournament (fp16 tensor_tensor runs 2x mode; reduce-accumulate is
always 1x so it only sees the last 1024 columns) plus one small
row-MIN over [0, 256) for the hardest positive. Host: sqrt / relu /
margin / exact label-count validity / mean. Simulated fp16 rel err
~1.8e-5 (gate 2e-3).
"""

import numpy as np

import concourse.bass as bass
import concourse.tile as tile
from concourse import bacc, mybir
from concourse.bass_utils import run_bass_kernel_spmd

B = 4096          # batch
D = 128           # embedding dim
NCORES = 8
R = B // NCORES   # rows per core (512)
MC = R // 128     # 128-row chunks per core (4)
W = 4096 + 3 * 128  # rotated table width: chunk m reads cols [m*128, m*128+4096)
NB = 512          # psum bank width at fp32
BAND = 32         # max |same-label col - row| after sorting (host-asserted)
MAXW = 256        # hardest-positive scan width (positives live in [17, 160+15])

BIGC = 2048.0     # same-label offset code (max d2 ~ 477)
MARGIN = 0.3
NJUNK = 6         # warm-up matmuls issued while the XT2 DMA streams

F32 = mybir.dt.float32
F16 = mybir.dt.float16
ALU = mybir.AluOpType
AXX = mybir.AxisListType.X

_CACHE: dict = {}


def build_nc() -> bass.Bass:
    nc = bacc.Bacc(None, target_bir_lowering=False)

    xt2 = nc.declare_dram_parameter("xt2", [D, W], F16, isOutput=False)
    packw = nc.declare_dram_parameter("packw", [2, W], F16, isOutput=False)  # -sq/2 hi|lo
    lh4 = nc.declare_dram_parameter("lh4", [128, MC * 128], F16, isOutput=False)
    rhs4 = nc.declare_dram_parameter("rhs4", [128, MC * NB], F16, isOutput=False)
    out = nc.declare_dram_parameter("out", [128, 2 * MC], F32, isOutput=True)

    with tile.TileContext(nc) as tc:
        with (
            tc.tile_pool(name="const", bufs=1) as cpool,
            tc.tile_pool(name="psum", bufs=1, space="PSUM") as psum,
            tc.tile_pool(name="evac", bufs=2) as epool,
            tc.tile_pool(name="work", bufs=1) as wpool,
        ):
            # XT2 bulk on the sync HWDGE ring; small tables on scalar's.
            XT2 = cpool.tile([D, W], F16)
            nc.sync.dma_start(XT2[:, 0:1024], xt2[:, 0:1024])
            nc.sync.dma_start(XT2[:, 1024:2752], xt2[:, 1024:2752])
            nc.sync.dma_start(XT2[:, 2752:W], xt2[:, 2752:W])
            LH4 = cpool.tile([128, MC * 128], F16)
            nc.scalar.dma_start(LH4[:], lh4[:])
            RHS4 = cpool.tile([128, MC * NB], F16)
            nc.gpsimd.dma_start(RHS4[:], rhs4[:])

            # Warm-up matmuls: pure junk, but they raise PE activity during
            # the DMA wait so the HAM clock-gate upshifts before real work.
            # Their memsets go first so the junk starts immediately.
            JW = cpool.tile([128, 128], F16)
            nc.vector.memset(JW[:], 0.0)
            JM = cpool.tile([128, NB], F16)
            nc.vector.memset(JM[:], 0.0)
            PJ = psum.tile([128, NB], F32, tag="PG1", name="pj")
            for _ in range(NJUNK):
                nc.tensor.matmul(PJ[:], JW[:], JM[:], start=True, stop=True)

            # PACKW: rows 96:97 carry -sq/2 hi/lo (matching LH4's 1.0 rows),
            # everything else zero so the dict rows of LH4 contribute nothing
            # on banks 1-7. Keeping every extra matmul at K=128 keeps PE
            # array utilization high enough for the HAM clock upshift.
            # Zero-fill through a uint32 view (fp16 memset runs 1x).
            PACKW = cpool.tile([128, W], F16)
            nc.vector.memset(PACKW.bitcast(mybir.dt.uint32), 0)
            nc.scalar.dma_start(PACKW[96:98, :], packw[:])

            OUT = wpool.tile([128, 2 * MC], F32)
            MA = wpool.tile([128, 1024], F16)
            MB = wpool.tile([128, 1024], F16)
            MCm = wpool.tile([128, 1024], F16)
            MD = wpool.tile([128, 512], F16)
            SC1 = wpool.tile([128, 512], F16)
            SC2 = wpool.tile([128, MAXW], F16)

            for m in range(MC):
                o = m * 128
                PG0 = psum.tile([128, 2560], F32, tag="PG0", name=f"pg0_{m}")
                PG1 = psum.tile([128, 1536], F32, tag="PG1", name=f"pg1_{m}")
                XS = XT2[:, 32 + o:32 + o + 128]   # chunk's own columns

                # Extras first (all with the LH4 stationary -> K=128 and a
                # single weight switch into the mains), mains last (XS).
                LHm = LH4[:, bass.ts(m, 128)]
                for b in range(1, 5):
                    nc.tensor.matmul(PG0[:, bass.ts(b, 512)], LHm,
                                     PACKW[:, o + b * 512:o + (b + 1) * 512],
                                     start=True, stop=False)
                for b in range(5, 8):
                    nc.tensor.matmul(PG1[:, bass.ts(b - 5, 512)], LHm,
                                     PACKW[:, o + b * 512:o + (b + 1) * 512],
                                     start=True, stop=False)
                nc.tensor.matmul(PG0[:, 0:512], LHm,
                                 RHS4[:, bass.ts(m, NB)],
                                 start=True, stop=False)
                for b in range(8):
                    if b < 5:
                        dst = PG0[:, bass.ts(b, 512)]
                    else:
                        dst = PG1[:, bass.ts(b - 5, 512)]
                    nc.tensor.matmul(dst, XS,
                                     XT2[:, o + b * 512:o + (b + 1) * 512],
                                     start=False, stop=True)

                # ScalarE evacuation to fp16 SBUF.
                F16E = epool.tile([128, 4096], F16, tag="f16e", name=f"f16e_{m}")
                nc.scalar.copy(F16E[:, 0:2560], PG0[:])
                nc.scalar.copy(F16E[:, 2560:4096], PG1[:])

                # DVE: hardest-neg = max v (tournament; fp16 TT runs 2x),
                # hardest-pos = min v over [0, 256) (masked sits at -BIG/2).
                nc.vector.tensor_tensor(MA[:], F16E[:, 0:1024],
                                        F16E[:, 1024:2048], op=ALU.max)
                nc.vector.tensor_scalar(
                    SC2[:], F16E[:, 0:MAXW], 0.0, None,
                    op0=ALU.add, op1=ALU.min,
                    accum_out=OUT[:, MC + m:MC + m + 1],
                )
                nc.vector.tensor_tensor(MB[:], F16E[:, 2048:3072],
                                        F16E[:, 3072:4096], op=ALU.max)
                nc.vector.tensor_tensor(MCm[:], MA[:], MB[:], op=ALU.max)
                nc.vector.tensor_tensor(MD[:], MCm[:, 0:512], MCm[:, 512:1024],
                                        op=ALU.max)
                nc.vector.tensor_scalar(
                    SC1[:], MD[:], 0.0, None,
                    op0=ALU.add, op1=ALU.max, accum_out=OUT[:, m:m + 1],
                )

            nc.sync.dma_start(out[:], OUT[:])

    nc.compile()
    return nc


def _get_nc() -> bass.Bass:
    if "nc" not in _CACHE:
        _CACHE["nc"] = build_nc()
    return _CACHE["nc"]


def prep_inputs(embeddings: np.ndarray, labels: np.ndarray):
    x = np.ascontiguousarray(np.asarray(embeddings, dtype=np.float32))
    lab0 = np.asarray(labels)

    # Sort the batch by label (loss is permutation invariant).
    perm = np.argsort(lab0, kind="stable")
    xs = x[perm]
    lab = lab0[perm].astype(np.int64)

    # Same-label columns of row g must lie within [g-BAND, g+BAND] so the
    # per-chunk mask window [0, 512) / positive window [0, 256) cover them.
    firsts: dict = {}
    lasts: dict = {}
    for i, l in enumerate(lab):
        if l not in firsts:
            firsts[l] = i
        lasts[l] = i
    idx = np.arange(B)
    first = np.array([firsts[l] for l in lab])
    last = np.array([lasts[l] for l in lab])
    assert (idx - first).max() <= BAND and (last - idx).max() <= BAND, \
        "label runs exceed the static positive window"

    xT = np.ascontiguousarray(xs.T)                      # [D, B] f32
    sq64 = np.einsum("ij,ij->i", xs.astype(np.float64), xs.astype(np.float64))
    sqh = sq64.astype(np.float16)
    sql = (sq64 - sqh.astype(np.float64)).astype(np.float16)
    nh = (-sqh / 2).astype(np.float16)                   # exact: /2 is a shift
    nl = (-sql / 2).astype(np.float16)

    in_maps = []
    for c in range(NCORES):
        rows = slice(c * R, (c + 1) * R)
        shift = BAND - c * R       # local col k <-> global (k + c*R - BAND) % B
        xb = np.roll(xT, shift, axis=1)
        xt2_c = np.concatenate([xb, xb[:, :W - B]], axis=1).astype(np.float16)
        nh2 = np.concatenate([np.roll(nh, shift), np.roll(nh, shift)[:W - B]])
        nl2 = np.concatenate([np.roll(nl, shift), np.roll(nl, shift)[:W - B]])
        labw = np.roll(lab, shift)
        packw_c = np.stack([nh2, nl2])

        lh4_c = np.zeros((128, MC * 128), np.float16)
        rhs4_c = np.zeros((128, MC * NB), np.float16)
        labo = lab[rows]
        for m in range(MC):
            u = np.unique(labo[m * 128:(m + 1) * 128])
            assert len(u) <= 96, f"chunk has {len(u)} distinct labels"
            lh4_c[0:len(u), m * 128:(m + 1) * 128] = (-BIGC / 2.0) * (
                u[:, None] == labo[None, m * 128:(m + 1) * 128])
            lh4_c[96:98, m * 128:(m + 1) * 128] = 1.0
            lw = labw[m * 128:m * 128 + NB]
            rhs4_c[0:len(u), m * NB:(m + 1) * NB] = (u[:, None] == lw[None, :])
            rhs4_c[96, m * NB:(m + 1) * NB] = nh2[m * 128:m * 128 + NB]
            rhs4_c[97, m * NB:(m + 1) * NB] = nl2[m * 128:m * 128 + NB]

        in_maps.append({
            "xt2": np.ascontiguousarray(xt2_c),
            "packw": np.ascontiguousarray(packw_c),
            "lh4": lh4_c, "rhs4": rhs4_c,
        })
    return in_maps, (lab, sq64)


def combine_outputs(results: list, lab: np.ndarray, sq64: np.ndarray) -> np.ndarray:
    mxv = np.empty(B)
    mnw = np.empty(B)
    for c, r in enumerate(results):
        o = np.asarray(r["out"], dtype=np.float64)       # [128, 8]
        for m in range(MC):
            g = slice(c * R + m * 128, c * R + (m + 1) * 128)
            mxv[g] = o[:, m]
            mnw[g] = o[:, MC + m]
    cnt = np.bincount(lab, minlength=int(lab.max()) + 1)[lab]
    valid = (cnt >= 2) & (cnt < B)
    hn2 = np.maximum(-2.0 * mxv + sq64, 0.0)
    hp2 = np.maximum(-2.0 * mnw - BIGC + sq64, 0.0)
    per = np.maximum(np.sqrt(hp2) - np.sqrt(hn2) + MARGIN, 0.0) * valid
    n_valid = valid.sum()
    val = per.sum() / max(n_valid, 1) if n_valid > 0 else 0.0
    return np.array(val, dtype=np.float32)


def run(embeddings: np.ndarray, labels: np.ndarray, **spmd_kwargs):
    nc = _get_nc()
    in_maps, (lab, sq64) = prep_inputs(embeddings, labels)
    res = run_bass_kernel_spmd(nc, in_maps, core_ids=list(range(NCORES)),
                               **spmd_kwargs)
    return combine_outputs(res.results, lab, sq64), res


def kernel(embeddings: np.ndarray, labels: np.ndarray) -> np.ndarray:
    loss, _ = run(embeddings, labels)
    return loss


# revision 15
# speedup vs baseline: 1.1637x; 1.1637x over previous
"""Batch-hard triplet loss on 8 Trainium2 NeuronCores.

Data-parallel over rows (512 rows/core, 4 chunks of 128). The batch is
label-sorted on the host and each (core, chunk) gets a rotated view of
the embedding table (local col j <-> global (j + c*512 - 32 + m*128)):
all same-label columns of chunk row p land in [p+32-B, p+32+B], B<=32
(host-asserted; actual ~15), so the mask matmul covers local cols
[0, 512) and the hardest-positive scan only cols [0, 256).

Per 128-row chunk the PE accumulates, in two 4-bank fp32 PSUM groups,
    v(p, j) = x_p . x_j - ||x_j||^2/2 - (BIG/2) * [lab_p == lab_j]
(so d2 = -2v + ||x_p||^2; the -2 and + ||x_p||^2 run on the host).
Matmuls are grouped by stationary operand - per chunk: 1 mask MM (LH4,
one-hot * -BIG/2 + norm rows), 7 K=2 norm MMs (ones @ -sq/2 hi/lo
rows), then 8 main MMs (stationary = the chunk's own 128 columns of
XT2) - only 2 weight switches, which keeps the PE dense so the HAM
clock-gate upshifts 1.2 -> 2.4 GHz (junk matmuls during the initial
DMA wait pre-warm it).

ScalarE evacuates each PSUM group to fp16 SBUF ([128,2048] ACTIVATE
copies, double-buffered); VectorE reduces the fp16 block with a MAX
tournament (fp16 tensor_tensor runs 2x mode; reduce-accumulate is
always 1x so it only sees the last 1024 columns) plus one small
row-MIN over [0, 256) for the hardest positive. Host: sqrt / relu /
margin / exact label-count validity / mean. Simulated fp16 rel err
~1.8e-5 (gate 2e-3).
"""

import numpy as np

import concourse.bass as bass
import concourse.tile as tile
from concourse import bacc, mybir
from concourse.bass_utils import run_bass_kernel_spmd

B = 4096          # batch
D = 128           # embedding dim
NCORES = 8
R = B // NCORES   # rows per core (512)
MC = R // 128     # 128-row chunks per core (4)
W = 4096 + 3 * 128  # rotated table width: chunk m reads cols [m*128, m*128+4096)
NB = 512          # psum bank width at fp32
BAND = 32         # max |same-label col - row| after sorting (host-asserted)
MAXW = 256        # hardest-positive scan width (positives live in [17, 160+15])

BIGC = 2048.0     # same-label offset code (max d2 ~ 477)
MARGIN = 0.3
NJUNK = 11        # warm-up matmuls issued while the XT2 DMA streams

F32 = mybir.dt.float32
F16 = mybir.dt.float16
ALU = mybir.AluOpType
AXX = mybir.AxisListType.X

_CACHE: dict = {}


def build_nc() -> bass.Bass:
    nc = bacc.Bacc(None, target_bir_lowering=False)

    xt2 = nc.declare_dram_parameter("xt2", [D, W], F16, isOutput=False)
    packw = nc.declare_dram_parameter("packw", [2, W], F16, isOutput=False)  # -sq/2 hi|lo
    lh4 = nc.declare_dram_parameter("lh4", [128, MC * 128], F16, isOutput=False)
    rhs4 = nc.declare_dram_parameter("rhs4", [128, MC * NB], F16, isOutput=False)
    out = nc.declare_dram_parameter("out", [128, 2 * MC], F32, isOutput=True)

    with tile.TileContext(nc) as tc:
        with (
            tc.tile_pool(name="const", bufs=1) as cpool,
            tc.tile_pool(name="psum", bufs=1, space="PSUM") as psum,
            tc.tile_pool(name="evac", bufs=2) as epool,
            tc.tile_pool(name="work", bufs=1) as wpool,
        ):
            # XT2 bulk on the sync HWDGE ring; small tables on scalar's.
            XT2 = cpool.tile([D, W], F16)
            nc.sync.dma_start(XT2[:, 0:1024], xt2[:, 0:1024])
            nc.sync.dma_start(XT2[:, 1024:2752], xt2[:, 1024:2752])
            nc.sync.dma_start(XT2[:, 2752:W], xt2[:, 2752:W])
            LH4 = cpool.tile([128, MC * 128], F16)
            nc.scalar.dma_start(LH4[:], lh4[:])
            RHS4 = cpool.tile([128, MC * NB], F16)
            nc.gpsimd.dma_start(RHS4[:], rhs4[:])

            # Warm-up matmuls: pure junk, but they raise PE activity during
            # the DMA wait so the HAM clock-gate upshifts before real work.
            # Their memsets go first so the junk starts immediately.
            JW = cpool.tile([128, 128], F16)
            nc.gpsimd.memset(JW[:], 0.0)
            JM = cpool.tile([128, NB], F16)
            nc.gpsimd.memset(JM[:], 0.0)
            PJ = psum.tile([128, NB], F32, tag="PG1", name="pj")
            for _ in range(NJUNK):
                nc.tensor.matmul(PJ[:], JW[:], JM[:], start=True, stop=True)

            # PACKW: rows 96:97 carry -sq/2 hi/lo (matching LH4's 1.0 rows),
            # everything else zero so the dict rows of LH4 contribute nothing
            # on banks 1-7. Keeping every extra matmul at K=128 keeps PE
            # array utilization high enough for the HAM clock upshift.
            # Zero-fill through a uint32 view (fp16 memset runs 1x).
            PACKW = cpool.tile([128, W], F16)
            nc.vector.memset(PACKW.bitcast(mybir.dt.uint32), 0)
            nc.scalar.dma_start(PACKW[96:98, :], packw[:])

            OUT = wpool.tile([128, 2 * MC], F32)
            MA = wpool.tile([128, 1024], F16)
            MB = wpool.tile([128, 1024], F16)
            MCm = wpool.tile([128, 1024], F16)
            MD = wpool.tile([128, 512], F16)
            SC1 = wpool.tile([128, 512], F16)
            SC2 = wpool.tile([128, MAXW], F16)

            for m in range(MC):
                o = m * 128
                PG0 = psum.tile([128, 2048], F32, tag="PG0", name=f"pg0_{m}")
                PG1 = psum.tile([128, 2048], F32, tag="PG1", name=f"pg1_{m}")
                XS = XT2[:, 32 + o:32 + o + 128]   # chunk's own columns

                # Extras first (all with the LH4 stationary -> K=128 and a
                # single weight switch into the mains), mains last (XS).
                LHm = LH4[:, bass.ts(m, 128)]
                for b in range(1, 4):
                    nc.tensor.matmul(PG0[:, bass.ts(b, 512)], LHm,
                                     PACKW[:, o + b * 512:o + (b + 1) * 512],
                                     start=True, stop=False)
                for b in range(4, 8):
                    nc.tensor.matmul(PG1[:, bass.ts(b - 4, 512)], LHm,
                                     PACKW[:, o + b * 512:o + (b + 1) * 512],
                                     start=True, stop=False)
                nc.tensor.matmul(PG0[:, 0:512], LHm,
                                 RHS4[:, bass.ts(m, NB)],
                                 start=True, stop=False)
                for b in range(8):
                    if b < 4:
                        dst = PG0[:, bass.ts(b, 512)]
                    else:
                        dst = PG1[:, bass.ts(b - 4, 512)]
                    nc.tensor.matmul(dst, XS,
                                     XT2[:, o + b * 512:o + (b + 1) * 512],
                                     start=False, stop=True)

                # ScalarE evacuation to fp16 SBUF.
                F16E = epool.tile([128, 4096], F16, tag="f16e", name=f"f16e_{m}")
                nc.scalar.copy(F16E[:, 0:2048], PG0[:])
                nc.scalar.copy(F16E[:, 2048:4096], PG1[:])

                # DVE: hardest-neg = max v (tournament; fp16 TT runs 2x),
                # hardest-pos = min v over [0, 256) (masked sits at -BIG/2).
                nc.vector.tensor_tensor(MA[:], F16E[:, 0:1024],
                                        F16E[:, 1024:2048], op=ALU.max)
                nc.vector.tensor_scalar(
                    SC2[:], F16E[:, 0:MAXW], 0.0, None,
                    op0=ALU.add, op1=ALU.min,
                    accum_out=OUT[:, MC + m:MC + m + 1],
                )
                nc.vector.tensor_tensor(MB[:], F16E[:, 2048:3072],
                                        F16E[:, 3072:4096], op=ALU.max)
                nc.vector.tensor_tensor(MCm[:], MA[:], MB[:], op=ALU.max)
                nc.vector.tensor_tensor(MD[:], MCm[:, 0:512], MCm[:, 512:1024],
                                        op=ALU.max)
                nc.vector.tensor_scalar(
                    SC1[:], MD[:], 0.0, None,
                    op0=ALU.add, op1=ALU.max, accum_out=OUT[:, m:m + 1],
                )

            nc.sync.dma_start(out[:], OUT[:])

    nc.compile()
    return nc


def _get_nc() -> bass.Bass:
    if "nc" not in _CACHE:
        _CACHE["nc"] = build_nc()
    return _CACHE["nc"]


def prep_inputs(embeddings: np.ndarray, labels: np.ndarray):
    x = np.ascontiguousarray(np.asarray(embeddings, dtype=np.float32))
    lab0 = np.asarray(labels)

    # Sort the batch by label (loss is permutation invariant).
    perm = np.argsort(lab0, kind="stable")
    xs = x[perm]
    lab = lab0[perm].astype(np.int64)

    # Same-label columns of row g must lie within [g-BAND, g+BAND] so the
    # per-chunk mask window [0, 512) / positive window [0, 256) cover them.
    firsts: dict = {}
    lasts: dict = {}
    for i, l in enumerate(lab):
        if l not in firsts:
            firsts[l] = i
        lasts[l] = i
    idx = np.arange(B)
    first = np.array([firsts[l] for l in lab])
    last = np.array([lasts[l] for l in lab])
    assert (idx - first).max() <= BAND and (last - idx).max() <= BAND, \
        "label runs exceed the static positive window"

    xT = np.ascontiguousarray(xs.T)                      # [D, B] f32
    sq64 = np.einsum("ij,ij->i", xs.astype(np.float64), xs.astype(np.float64))
    sqh = sq64.astype(np.float16)
    sql = (sq64 - sqh.astype(np.float64)).astype(np.float16)
    nh = (-sqh / 2).astype(np.float16)                   # exact: /2 is a shift
    nl = (-sql / 2).astype(np.float16)

    in_maps = []
    for c in range(NCORES):
        rows = slice(c * R, (c + 1) * R)
        shift = BAND - c * R       # local col k <-> global (k + c*R - BAND) % B
        xb = np.roll(xT, shift, axis=1)
        xt2_c = np.concatenate([xb, xb[:, :W - B]], axis=1).astype(np.float16)
        nh2 = np.concatenate([np.roll(nh, shift), np.roll(nh, shift)[:W - B]])
        nl2 = np.concatenate([np.roll(nl, shift), np.roll(nl, shift)[:W - B]])
        labw = np.roll(lab, shift)
        packw_c = np.stack([nh2, nl2])

        lh4_c = np.zeros((128, MC * 128), np.float16)
        rhs4_c = np.zeros((128, MC * NB), np.float16)
        labo = lab[rows]
        for m in range(MC):
            u = np.unique(labo[m * 128:(m + 1) * 128])
            assert len(u) <= 96, f"chunk has {len(u)} distinct labels"
            lh4_c[0:len(u), m * 128:(m + 1) * 128] = (-BIGC / 2.0) * (
                u[:, None] == labo[None, m * 128:(m + 1) * 128])
            lh4_c[96:98, m * 128:(m + 1) * 128] = 1.0
            lw = labw[m * 128:m * 128 + NB]
            rhs4_c[0:len(u), m * NB:(m + 1) * NB] = (u[:, None] == lw[None, :])
            rhs4_c[96, m * NB:(m + 1) * NB] = nh2[m * 128:m * 128 + NB]
            rhs4_c[97, m * NB:(m + 1) * NB] = nl2[m * 128:m * 128 + NB]

        in_maps.append({
            "xt2": np.ascontiguousarray(xt2_c),
            "packw": np.ascontiguousarray(packw_c),
            "lh4": lh4_c, "rhs4": rhs4_c,
        })
    return in_maps, (lab, sq64)


def combine_outputs(results: list, lab: np.ndarray, sq64: np.ndarray) -> np.ndarray:
    mxv = np.empty(B)
    mnw = np.empty(B)
    for c, r in enumerate(results):
        o = np.asarray(r["out"], dtype=np.float64)       # [128, 8]
        for m in range(MC):
            g = slice(c * R + m * 128, c * R + (m + 1) * 128)
            mxv[g] = o[:, m]
            mnw[g] = o[:, MC + m]
    cnt = np.bincount(lab, minlength=int(lab.max()) + 1)[lab]
    valid = (cnt >= 2) & (cnt < B)
    hn2 = np.maximum(-2.0 * mxv + sq64, 0.0)
    hp2 = np.maximum(-2.0 * mnw - BIGC + sq64, 0.0)
    per = np.maximum(np.sqrt(hp2) - np.sqrt(hn2) + MARGIN, 0.0) * valid
    n_valid = valid.sum()
    val = per.sum() / max(n_valid, 1) if n_valid > 0 else 0.0
    return np.array(val, dtype=np.float32)


def run(embeddings: np.ndarray, labels: np.ndarray, **spmd_kwargs):
    nc = _get_nc()
    in_maps, (lab, sq64) = prep_inputs(embeddings, labels)
    res = run_bass_kernel_spmd(nc, in_maps, core_ids=list(range(NCORES)),
                               **spmd_kwargs)
    return combine_outputs(res.results, lab, sq64), res


def kernel(embeddings: np.ndarray, labels: np.ndarray) -> np.ndarray:
    loss, _ = run(embeddings, labels)
    return loss


# revision 17
# speedup vs baseline: 1.2117x; 1.0413x over previous
"""Batch-hard triplet loss on 8 Trainium2 NeuronCores.

Data-parallel over rows (512 rows/core, 4 chunks of 128). The batch is
label-sorted on the host and each (core, chunk) gets a rotated view of
the embedding table (local col j <-> global (j + c*512 - 32 + m*128)):
all same-label columns of chunk row p land in [p+32-B, p+32+B], B<=32
(host-asserted; actual ~15), so the mask matmul covers local cols
[0, 512) and the hardest-positive scan only cols [0, 256).

Per 128-row chunk the PE accumulates, in two 4-bank fp32 PSUM groups,
    v(p, j) = x_p . x_j - ||x_j||^2/2 - (BIG/2) * [lab_p == lab_j]
(so d2 = -2v + ||x_p||^2; the -2 and + ||x_p||^2 run on the host).
Matmuls are grouped by stationary operand - per chunk: 8 extras with
the LH4 stationary (bank 0: mask one-hot * -BIG/2 + norm rows; banks
1-7: PACKW, whose only nonzero rows 96:97 carry -sq/2 hi/lo and meet
LH4's 1.0 rows), then 8 mains (stationary = the chunk's own columns of
XT2). Keeping every matmul at K=128 with only 2 weight switches per
chunk keeps PE array utilization high enough that the HAM clock-gate
upshifts 1.2 -> 2.4 GHz; junk matmuls bridge the initial DMA wait so
the array is already warm when real work arrives. Chunk 0 runs mains
before extras (its mask tables land after XT2's leading blocks).

ScalarE evacuates each PSUM group to fp16 SBUF ([128,2048] ACTIVATE
copies, double-buffered); VectorE reduces the fp16 block with a MAX
tournament (fp16 tensor_tensor runs 2x mode; reduce-accumulate is
always 1x so it only sees the last 512 columns) plus one small
row-MIN over [0, 256) for the hardest positive. The last chunk splits
its second evacuation and uses a finer tree so the drain tail is
short. Host: sqrt / relu / margin / exact label-count validity / mean.
Simulated fp16 rel err ~1.8e-5 (gate 2e-3).
"""

import numpy as np

import concourse.bass as bass
import concourse.tile as tile
from concourse import bacc, mybir
from concourse.bass_utils import run_bass_kernel_spmd

B = 4096          # batch
D = 128           # embedding dim
NCORES = 8
R = B // NCORES   # rows per core (512)
MC = R // 128     # 128-row chunks per core (4)
W = 4096 + 3 * 128  # rotated table width: chunk m reads cols [m*128, m*128+4096)
NB = 512          # psum bank width at fp32
BAND = 32         # max |same-label col - row| after sorting (host-asserted)
MAXW = 256        # hardest-positive scan width (positives live in [17, 160+15])
NDICT = 32        # dict slots per chunk (host-asserted)

BIGC = 2048.0     # same-label offset code (max d2 ~ 477)
MARGIN = 0.3
NJUNK = 14        # warm-up matmuls issued while the XT2 DMA streams

F32 = mybir.dt.float32
F16 = mybir.dt.float16
ALU = mybir.AluOpType
AXX = mybir.AxisListType.X

_CACHE: dict = {}


def build_nc() -> bass.Bass:
    nc = bacc.Bacc(None, target_bir_lowering=False)

    xt2 = nc.declare_dram_parameter("xt2", [D, W], F16, isOutput=False)
    packw = nc.declare_dram_parameter("packw", [2, W], F16, isOutput=False)  # -sq/2 hi|lo
    lh4 = nc.declare_dram_parameter("lh4", [128, MC * 128], F16, isOutput=False)
    rhs4 = nc.declare_dram_parameter("rhs4", [NDICT + 2, MC * NB], F16,
                                     isOutput=False)
    out = nc.declare_dram_parameter("out", [128, 2 * MC], F32, isOutput=True)

    with tile.TileContext(nc) as tc:
        with (
            tc.tile_pool(name="const", bufs=1) as cpool,
            tc.tile_pool(name="psum", bufs=1, space="PSUM") as psum,
            tc.tile_pool(name="evac", bufs=2) as epool,
            tc.tile_pool(name="work", bufs=1) as wpool,
        ):
            # XT2 bulk on the sync HWDGE ring; small tables on scalar's.
            XT2 = cpool.tile([D, W], F16)
            nc.sync.dma_start(XT2[:, 0:1024], xt2[:, 0:1024])
            nc.sync.dma_start(XT2[:, 1024:2752], xt2[:, 1024:2752])
            nc.sync.dma_start(XT2[:, 2752:W], xt2[:, 2752:W])

            # Warm-up matmuls: pure junk, but they raise PE activity during
            # the DMA wait so the HAM clock-gate upshifts before real work.
            # Their memsets go first (gpsimd) so the junk starts immediately.
            JW = cpool.tile([128, 128], F16)
            nc.gpsimd.memset(JW[:], 0.0)
            JM = cpool.tile([128, NB], F16)
            nc.gpsimd.memset(JM[:], 0.0)
            PJ = psum.tile([128, NB], F32, tag="PG1", name="pj")
            for _ in range(NJUNK):
                nc.tensor.matmul(PJ[:], JW[:], JM[:], start=True, stop=True)

            LH4 = cpool.tile([128, MC * 128], F16)
            nc.scalar.dma_start(LH4[:], lh4[:])

            # PACKW: rows 96:97 carry -sq/2 hi/lo (matching LH4's 1.0 rows),
            # everything else zero so the dict rows of LH4 contribute nothing
            # on banks 1-7. Zero-fill through a uint32 view (fp16 memset
            # runs 1x). RHS4 ships compressed: 32 dict rows + 2 norm rows.
            PACKW = cpool.tile([128, W], F16)
            nc.vector.memset(PACKW.bitcast(mybir.dt.uint32), 0)
            nc.scalar.dma_start(PACKW[96:98, :], packw[:])
            RHS4 = cpool.tile([128, MC * NB], F16)
            nc.vector.memset(RHS4.bitcast(mybir.dt.uint32), 0)
            nc.gpsimd.dma_start(RHS4[0:NDICT, :], rhs4[0:NDICT, :])
            nc.gpsimd.dma_start(RHS4[96:98, :], rhs4[NDICT:NDICT + 2, :])

            OUT = wpool.tile([128, 2 * MC], F32)
            MA = wpool.tile([128, 1024], F16)
            MB = wpool.tile([128, 1024], F16)
            MCm = wpool.tile([128, 1024], F16)
            MD = wpool.tile([128, 512], F16)
            ME = wpool.tile([128, 512], F16)
            MF = wpool.tile([128, 512], F16)
            SC1 = wpool.tile([128, 512], F16)
            SC2 = wpool.tile([128, MAXW], F16)

            for m in range(MC):
                o = m * 128
                PG0 = psum.tile([128, 2048], F32, tag="PG0", name=f"pg0_{m}")
                PG1 = psum.tile([128, 2048], F32, tag="PG1", name=f"pg1_{m}")
                XS = XT2[:, 32 + o:32 + o + 128]   # chunk's own columns
                LHm = LH4[:, bass.ts(m, 128)]

                def mains(start, stop):
                    for b in range(8):
                        if b < 4:
                            dst = PG0[:, bass.ts(b, 512)]
                        else:
                            dst = PG1[:, bass.ts(b - 4, 512)]
                        nc.tensor.matmul(dst, XS,
                                         XT2[:, o + b * 512:o + (b + 1) * 512],
                                         start=start, stop=stop)

                def extras(start, stop):
                    for b in range(1, 4):
                        nc.tensor.matmul(PG0[:, bass.ts(b, 512)], LHm,
                                         PACKW[:, o + b * 512:o + (b + 1) * 512],
                                         start=start, stop=stop)
                    for b in range(4, 8):
                        nc.tensor.matmul(PG1[:, bass.ts(b - 4, 512)], LHm,
                                         PACKW[:, o + b * 512:o + (b + 1) * 512],
                                         start=start, stop=stop)
                    nc.tensor.matmul(PG0[:, 0:512], LHm,
                                     RHS4[:, bass.ts(m, NB)],
                                     start=start, stop=stop)

                if m == 0:
                    # Chunk 0: mains first (mask tables land after XT2's
                    # leading blocks); accumulation order is irrelevant.
                    mains(True, False)
                    extras(False, True)
                else:
                    extras(True, False)
                    mains(False, True)

                F16E = epool.tile([128, 4096], F16, tag="f16e", name=f"f16e_{m}")
                nc.scalar.copy(F16E[:, 0:2048], PG0[:])
                if m < MC - 1:
                    nc.scalar.copy(F16E[:, 2048:4096], PG1[:])
                    # DVE: hardest-neg = max v (fp16 TT tournament at 2x),
                    # hardest-pos = min v over [0, 256).
                    nc.vector.tensor_tensor(MA[:], F16E[:, 0:1024],
                                            F16E[:, 1024:2048], op=ALU.max)
                    nc.vector.tensor_scalar(
                        SC2[:], F16E[:, 0:MAXW], 0.0, None,
                        op0=ALU.add, op1=ALU.min,
                        accum_out=OUT[:, MC + m:MC + m + 1],
                    )
                    nc.vector.tensor_tensor(MB[:], F16E[:, 2048:3072],
                                            F16E[:, 3072:4096], op=ALU.max)
                    nc.vector.tensor_tensor(MCm[:], MA[:], MB[:], op=ALU.max)
                    nc.vector.tensor_tensor(MD[:], MCm[:, 0:512],
                                            MCm[:, 512:1024], op=ALU.max)
                    nc.vector.tensor_scalar(
                        SC1[:], MD[:], 0.0, None,
                        op0=ALU.add, op1=ALU.max, accum_out=OUT[:, m:m + 1],
                    )
                else:
                    # Last chunk: split the final evacuation and use a finer
                    # tree so the post-evacuation drain is short.
                    nc.scalar.copy(F16E[:, 2048:3072], PG1[:, 0:1024])
                    nc.scalar.copy(F16E[:, 3072:4096], PG1[:, 1024:2048])
                    nc.vector.tensor_tensor(MA[:], F16E[:, 0:1024],
                                            F16E[:, 1024:2048], op=ALU.max)
                    nc.vector.tensor_scalar(
                        SC2[:], F16E[:, 0:MAXW], 0.0, None,
                        op0=ALU.add, op1=ALU.min,
                        accum_out=OUT[:, MC + m:MC + m + 1],
                    )
                    nc.vector.tensor_tensor(MD[:], MA[:, 0:512],
                                            MA[:, 512:1024], op=ALU.max)
                    nc.vector.tensor_tensor(ME[:], F16E[:, 2048:2560],
                                            F16E[:, 2560:3072], op=ALU.max)
                    nc.vector.tensor_tensor(ME[:], MD[:], ME[:], op=ALU.max)
                    nc.vector.tensor_tensor(MF[:], F16E[:, 3072:3584],
                                            F16E[:, 3584:4096], op=ALU.max)
                    nc.vector.tensor_tensor(MF[:], ME[:], MF[:], op=ALU.max)
                    nc.vector.tensor_scalar(
                        SC1[:], MF[:], 0.0, None,
                        op0=ALU.add, op1=ALU.max, accum_out=OUT[:, m:m + 1],
                    )

            nc.sync.dma_start(out[:], OUT[:])

    nc.compile()
    return nc


def _get_nc() -> bass.Bass:
    if "nc" not in _CACHE:
        _CACHE["nc"] = build_nc()
    return _CACHE["nc"]


def prep_inputs(embeddings: np.ndarray, labels: np.ndarray):
    x = np.ascontiguousarray(np.asarray(embeddings, dtype=np.float32))
    lab0 = np.asarray(labels)

    # Sort the batch by label (loss is permutation invariant).
    perm = np.argsort(lab0, kind="stable")
    xs = x[perm]
    lab = lab0[perm].astype(np.int64)

    # Same-label columns of row g must lie within [g-BAND, g+BAND] so the
    # per-chunk mask window [0, 512) / positive window [0, 256) cover them.
    firsts: dict = {}
    lasts: dict = {}
    for i, l in enumerate(lab):
        if l not in firsts:
            firsts[l] = i
        lasts[l] = i
    idx = np.arange(B)
    first = np.array([firsts[l] for l in lab])
    last = np.array([lasts[l] for l in lab])
    assert (idx - first).max() <= BAND and (last - idx).max() <= BAND, \
        "label runs exceed the static positive window"

    xT = np.ascontiguousarray(xs.T)                      # [D, B] f32
    sq64 = np.einsum("ij,ij->i", xs.astype(np.float64), xs.astype(np.float64))
    sqh = sq64.astype(np.float16)
    sql = (sq64 - sqh.astype(np.float64)).astype(np.float16)
    nh = (-sqh / 2).astype(np.float16)                   # exact: /2 is a shift
    nl = (-sql / 2).astype(np.float16)

    in_maps = []
    for c in range(NCORES):
        rows = slice(c * R, (c + 1) * R)
        shift = BAND - c * R       # local col k <-> global (k + c*R - BAND) % B
        xb = np.roll(xT, shift, axis=1)
        xt2_c = np.concatenate([xb, xb[:, :W - B]], axis=1).astype(np.float16)
        nh2 = np.concatenate([np.roll(nh, shift), np.roll(nh, shift)[:W - B]])
        nl2 = np.concatenate([np.roll(nl, shift), np.roll(nl, shift)[:W - B]])
        labw = np.roll(lab, shift)
        packw_c = np.stack([nh2, nl2])

        lh4_c = np.zeros((128, MC * 128), np.float16)
        rhs4_c = np.zeros((NDICT + 2, MC * NB), np.float16)
        labo = lab[rows]
        for m in range(MC):
            u = np.unique(labo[m * 128:(m + 1) * 128])
            assert len(u) <= NDICT, f"chunk has {len(u)} distinct labels"
            lh4_c[0:len(u), m * 128:(m + 1) * 128] = (-BIGC / 2.0) * (
                u[:, None] == labo[None, m * 128:(m + 1) * 128])
            lh4_c[96:98, m * 128:(m + 1) * 128] = 1.0
            lw = labw[m * 128:m * 128 + NB]
            rhs4_c[0:len(u), m * NB:(m + 1) * NB] = (u[:, None] == lw[None, :])
            rhs4_c[NDICT, m * NB:(m + 1) * NB] = nh2[m * 128:m * 128 + NB]
            rhs4_c[NDICT + 1, m * NB:(m + 1) * NB] = nl2[m * 128:m * 128 + NB]

        in_maps.append({
            "xt2": np.ascontiguousarray(xt2_c),
            "packw": np.ascontiguousarray(packw_c),
            "lh4": lh4_c, "rhs4": rhs4_c,
        })
    return in_maps, (lab, sq64)


def combine_outputs(results: list, lab: np.ndarray, sq64: np.ndarray) -> np.ndarray:
    mxv = np.empty(B)
    mnw = np.empty(B)
    for c, r in enumerate(results):
        o = np.asarray(r["out"], dtype=np.float64)       # [128, 8]
        for m in range(MC):
            g = slice(c * R + m * 128, c * R + (m + 1) * 128)
            mxv[g] = o[:, m]
            mnw[g] = o[:, MC + m]
    cnt = np.bincount(lab, minlength=int(lab.max()) + 1)[lab]
    valid = (cnt >= 2) & (cnt < B)
    hn2 = np.maximum(-2.0 * mxv + sq64, 0.0)
    hp2 = np.maximum(-2.0 * mnw - BIGC + sq64, 0.0)
    per = np.maximum(np.sqrt(hp2) - np.sqrt(hn2) + MARGIN, 0.0) * valid
    n_valid = valid.sum()
    val = per.sum() / max(n_valid, 1) if n_valid > 0 else 0.0
    return np.array(val, dtype=np.float32)


def run(embeddings: np.ndarray, labels: np.ndarray, **spmd_kwargs):
    nc = _get_nc()
    in_maps, (lab, sq64) = prep_inputs(embeddings, labels)
    res = run_bass_kernel_spmd(nc, in_maps, core_ids=list(range(NCORES)),
                               **spmd_kwargs)
    return combine_outputs(res.results, lab, sq64), res


def kernel(embeddings: np.ndarray, labels: np.ndarray) -> np.ndarray:
    loss, _ = run(embeddings, labels)
    return loss
